# revision 52
# baseline (speedup 1.0000x reference)
"""Causal self-attention (single-head, d_model=512) on 8 Trainium2 cores.

Problem: x[4,4096,512] fp32, w_qkv[1536,512], w_proj[512,512]
  qkv = x @ w_qkv.T; scores = q k^T / sqrt(512) causal-masked; softmax;
  out = (softmax @ v) @ w_proj.T

Weight folding (host-side, free): scores^T = k q^T
  = x (Wk^T Wq / sqrt(C)) xq^T = x (M xq^T), so the kernel transforms
  only the 2048 query rows (mq = M xq^T) and uses raw x tiles as the
  stationary operand of the scores matmul -- no key transform at all.
  Likewise y = softmax(..) v Wp^T = D^{-1} (E^T x) (Wp Wv)^T, so
  S = E^T x is accumulated directly from x tiles and the projection
  applies Wpv = Wp Wv (v production gone too).

Sharding: 2 cores per batch; rows assigned at 128-row tile granularity.
Row-tile t (rows 128t..) needs k-tiles 0..t (extent t+1). Part 0 takes
odd tiles {1,3,..,31} (extents 2,4,..,32), part 1 even tiles (extents
one less). Both parts share one baked per-slot extent profile (the
max): slot i holds 4 tiles with profile extents P0-2j laid out in
DECREASING extent order along the 512-row slot, so the k-loop narrows
its matmul width from 512 to 128 as k passes each tile's extent.
Attention work: 272 key-tile units/core vs 320 for rectangular blocks.

Layouts avoid every on-chip transpose:
  scores^T[key,row] = (x^T chunk).T @ (mq chunk)  -- both [C,*] layouts
  S[cx,row]        += (x tile).T @ exp(scores^T)  -- x natural layout
  y[row,o]          = (S chunk).T @ WpvT chunk
Row-sums: e tiles are accumulated into esum[key,row] on the GpSimd
engine, then one tiny matmul per 128-row group (lhsT=esum chunk,
rhs=ones) yields rowsums directly in row-partition layout [128,1].
Softmax skips max-subtraction (scores are ~N(0,1); exp is safe in fp32).

Scheduling (the difference between 189us and 171us):
  - inputs host-pre-tiled to exact SBUF layouts; critical transfers
    (m/qx) split small across both HW DMA rings (SP+Act), bulk on SP;
  - dummy warm-up matmuls during the initial DMA wait keep the PE
    clock-gate (HAM) at 2.4 GHz for phase B;
  - two-deep score software pipeline hides the exp->mask->S chain;
  - causal masks precomputed on host (bm input), applied as one DVE
    multiply on the single boundary tile per key-step;
  - each slot's projection is deferred into the next slot so PSUM
    evacuation copies never stall the in-order PE queue;
  - projection PSUM rides the score-tile rotation (yp double-buffered
    for free), rowsums emitted early per rt.
"""

import numpy as np
import ml_dtypes
from contextlib import ExitStack

import concourse.bass as bass
import concourse.mybir as mybir
import concourse.tile as tile

B, T, C = 4, 4096, 512
NCORES = 8
QR = 2048  # rows per core (16 row-tiles of 128)
CC = C // 128  # contraction chunks (4)
TK = T // 128  # key tiles (32)
NSLOT = 4
P0S = [8, 16, 24, 32]  # slot profile base: slot i tiles have extents P0-2j
SCALE = 1.0 / np.sqrt(C)

BF = mybir.dt.bfloat16
F32 = mybir.dt.float32
BFNP = ml_dtypes.bfloat16

# part -> slot(i, by P0S order) -> 4 global row-tile ids, decreasing extent.
# part 0: tile P0-2j-1 (extent P0-2j = profile); part 1: one less.
ASSIGN_TILES = {
    0: [[P0 - 2 * j - 1 for j in range(4)] for P0 in P0S],
    1: [[P0 - 2 * j - 2 for j in range(4)] for P0 in P0S],
}


def _width(P0, k):
    """number of 128-row tiles still active at key-tile k (1..4)"""
    return min(4, (P0 - k + 1) // 2)


def _strip_mm_sem_incs(nc):
    """Every matmul carries a PE-semaphore increment, and each increment
    costs ~26ns of PE issue time (EVT_SEM register write) -- ~20us across
    ~800 matmuls. Keep the increment only on accumulation-group finals
    (stop=True; the only counts consumers genuinely need) and remap every
    wait on that semaphore: a wait on a mid-group count rounds up to the
    next kept increment. PE completes matmuls strictly in order, so waiting
    for a later matmul is always correct, merely (slightly) later."""
    mms = []
    for _, bbb in nc.bb_map.items():
        for inst in bbb.bb.instructions:
            if isinstance(inst, mybir.InstMatmult):
                mms.append(inst)
    if not mms:
        return
    sem_names = set()
    for mm in mms:
        si = mm.sync_info
        if si and si.on_update:
            for u in si.on_update:
                if u.update_mode == "sem-inc":
                    sem_names.add(u.ant_name)
    assert len(sem_names) == 1, sem_names
    sem = sem_names.pop()

    keep = [bool(mm.stop_tensor_calc) for mm in mms]
    keep[-1] = True
    # prefix[v] = kept increments among the first v matmuls
    prefix = [0]
    for kp in keep:
        prefix.append(prefix[-1] + (1 if kp else 0))
    total = prefix[-1]

    def remap(v):
        assert 1 <= v <= len(mms), v
        nv = prefix[v] if keep[v - 1] else prefix[v] + 1
        return min(nv, total)

    for _, bbb in nc.bb_map.items():
        for inst in bbb.bb.instructions:
            si = inst.sync_info
            if si and si.on_wait:
                for w in si.on_wait:
                    if w.ant_name == sem and w.wait_mode == "sem-ge-imm":
                        w.wait_value = remap(w.wait_value)
    for mm, kp in zip(mms, keep):
        if not kp:
            si = mm.sync_info
            si.on_update = [
                u for u in si.on_update if u.ant_name != sem
            ]


def _split_excess_waits(nc, max_waits=1):
    """The walrus build in this env rejects >1 sync-wait command on one
    instruction; hoist extras onto standalone same-engine NoOps."""
    for _, bbb in nc.bb_map.items():
        bb = bbb.bb
        new = []
        for inst in list(bb.instructions):
            si = inst.sync_info
            waits = list(si.on_wait) if si and si.on_wait else []
            if len(waits) > max_waits:
                for j, w in enumerate(waits[max_waits:]):
                    new.append(
                        mybir.InstNoOp(
                            name=f"{inst.name}-hw{j}",
                            engine=inst.engine,
                            sync_info=mybir.SyncInfo(on_wait=[w], on_update=[]),
                        )
                    )
                si.on_wait = waits[:max_waits]
                inst.sync_info = si
            new.append(inst)
        bb.instructions = new


def build_program():
    nc = bass.Bass()
    # all inputs are host-pre-tiled into their exact SBUF layouts so every
    # DMA has long contiguous per-partition runs (2-16 KiB descriptors)
    d_xT = nc.dram_tensor("xTt", [128, T // 512, CC * 512], BF, kind="ExternalInput")
    d_xN = nc.dram_tensor("xNt", [128, TK, C], BF, kind="ExternalInput")
    # m and qxT ride in one tensor so the phase-B critical path is a single
    # DMA (per-transfer descriptor overhead dominates small transfers):
    # [:, 0:2048] = m pre-tiled, [:, 2048+qb*2048+cc*512+t] = qx chunks
    d_mqx = nc.dram_tensor("mqx", [128, 2048 + QR * CC], BF, kind="ExternalInput")
    d_wpv = nc.dram_tensor("wpvTt", [128, CC, C], BF, kind="ExternalInput")
    # per-core boundary masks: for each slot i, the last 8 key-tiles have
    # exactly one masked 128-row chunk (the lowest-extent active tile);
    # bm[:, i*8+bk, :] is that {0,1} mask, precomputed on host (it differs
    # between part-0 and part-1 cores, which is what keeps the program SPMD).
    d_bm = nc.dram_tensor("bm", [128, NSLOT * 8 * 128], BF, kind="ExternalInput")
    d_y = nc.dram_tensor("y", [QR, C], F32, kind="ExternalOutput")

    with tile.TileContext(nc) as tc:
        with ExitStack() as ctx:
            const = ctx.enter_context(tc.tile_pool(name="const", bufs=1))
            work = ctx.enter_context(tc.tile_pool(name="work", bufs=3))

            # ---- persistent SBUF tensors (layouts match the pre-tiled
            # HBM inputs exactly, so each DMA is long contiguous runs) ----
            xts = const.tile([128, T // 512, CC * 512], BF, tag="xts")
            xn = const.tile([128, TK, C], BF, tag="xn")
            mqx = const.tile([128, 2048 + QR * CC], BF, tag="mqx")
            wpv = const.tile([128, CC, C], BF, tag="wpv")
            bm = const.tile([128, NSLOT * 8, 128], BF, tag="bm")
            mq = const.tile([128, CC, QR], BF, tag="mq")  # (M xq^T)[cx, row]
            ones = const.tile([128, 1], F32, tag="ones")
            esum = const.tile([128, 512], F32, tag="esum")
            rr = const.tile([128, 16], F32, tag="rr")  # 1/rowsum, [p, slot*4+rt]

            warm = const.tile([128, 512], BF, tag="warm")

            xT_r = d_xT.ap()
            xN_r = d_xN.ap()
            # DMA ring plan: only SP (sync) and Act (scalar) have hardware
            # DGE rings (gpsimd is the slow software DGE -- unused). Each
            # ring drains its queue in order, rings share HBM bandwidth.
            # dma_start occupies the issuing engine ~2us/MiB, so the Act
            # ring carries only the small critical transfers -- its queue
            # must reach the phase-B mq copies quickly. SP is otherwise
            # idle until the output DMAs and takes the bulk in need-order.
            nc.gpsimd.memset(warm[:], 1.0)
            nc.gpsimd.memset(ones[:], 1.0)
            # critical path (m + qx chunk 0, 1 MiB) as four 0.25 MiB
            # transfers alternating rings: descriptors are serial within a
            # transfer but pipeline across transfers and rings. qxt chunks
            # 1/2/3 split across rings so no phase-B group waits long.
            nc.scalar.dma_start(mqx[:, 0:1024], d_mqx.ap()[:, 0:1024])
            nc.sync.dma_start(mqx[:, 2048:3072], d_mqx.ap()[:, 2048:3072])
            nc.scalar.dma_start(mqx[:, 3072:4096], d_mqx.ap()[:, 3072:4096])
            nc.sync.dma_start(mqx[:, 1024:2048], d_mqx.ap()[:, 1024:2048])
            nc.scalar.dma_start(mqx[:, 4096:6144], d_mqx.ap()[:, 4096:6144])
            nc.sync.dma_start(mqx[:, 6144:8192], d_mqx.ap()[:, 6144:8192])
            nc.sync.dma_start(mqx[:, 8192:10240], d_mqx.ap()[:, 8192:10240])
            nc.scalar.dma_start(
                bm[:], d_bm.ap().rearrange("p (g c) -> p g c", c=128)
            )
            nc.sync.dma_start(xts[:, 0:1, :], xT_r[:, 0:1, :])
            nc.sync.dma_start(xn[:, 0:4, :], xN_r[:, 0:4, :])
            nc.sync.dma_start(xts[:, 1:2, :], xT_r[:, 1:2, :])
            nc.sync.dma_start(xn[:, 4:8, :], xN_r[:, 4:8, :])
            nc.sync.dma_start(wpv[:], d_wpv.ap())
            nc.sync.dma_start(xts[:, 2:3, :], xT_r[:, 2:3, :])
            nc.sync.dma_start(xts[:, 3:4, :], xT_r[:, 3:4, :])
            nc.sync.dma_start(xn[:, 8:16, :], xN_r[:, 8:16, :])
            nc.sync.dma_start(xts[:, 4:5, :], xT_r[:, 4:5, :])
            nc.sync.dma_start(xts[:, 5:6, :], xT_r[:, 5:6, :])
            nc.sync.dma_start(xn[:, 16:24, :], xN_r[:, 16:24, :])
            nc.sync.dma_start(xts[:, 6:7, :], xT_r[:, 6:7, :])
            nc.sync.dma_start(xts[:, 7:8, :], xT_r[:, 7:8, :])
            nc.sync.dma_start(xn[:, 24:TK, :], xN_r[:, 24:TK, :])

            # ---- ONE PSUM pool for the whole kernel: closing a pool
            # between phase B and attention gates the next pool's first
            # allocations on ALL of the old pool's readers (the 16 mq
            # copies) -- an ~800ns PE stall. Instead the warm-up rides the
            # rowsum bank and phase B rides the score-tile rotation; the
            # copies lag each group by ~720ns, well inside the 3-bank
            # rotation depth, so nothing waits. ----
            with tc.tile_pool(name="ps_at", bufs=1, space="PSUM") as ps_at:
                # HAM warm-up: PE idles a few us waiting for m/qxt to land;
                # scratch matmuls keep the clock-gate busy so phase B and
                # early attention run at 2.4 GHz instead of 1.2.
                wps = ps_at.tile([128, 512], F32, tag="rs", bufs=1, name="wps")
                for _ in range(8):
                    nc.tensor.matmul(
                        wps[:], lhsT=warm[:, 0:128], rhs=warm[:],
                        start=True, stop=True,
                    )
                # phase B: mq = M xq^T (query transform; qb-outer so the
                # first attention slot's chunk completes first)
                for qb in range(QR // 512):
                    for oc in range(CC):
                        ps = ps_at.tile([128, 512], F32, tag="st", bufs=3, name="ps")
                        for cc in range(CC):
                            o0 = (oc // 2) * 1024 + cc * 256 + (oc % 2) * 128
                            q0 = 2048 + qb * 2048 + cc * 512
                            nc.tensor.matmul(
                                ps[:],
                                lhsT=mqx[:, o0 : o0 + 128],
                                rhs=mqx[:, q0 : q0 + 512],
                                start=(cc == 0),
                                stop=(cc == CC - 1),
                            )
                        # alternate evacuation engines: 16 serial 720ns
                        # copies on ACT alone lag phase B and stall the
                        # early slots' score matmuls
                        dst = mq[:, oc, qb * 512 : (qb + 1) * 512]
                        if (qb * CC + oc) % 2 == 0:
                            nc.scalar.copy(dst, ps[:])
                        else:
                            nc.vector.tensor_copy(dst, ps[:])

                # phases C+D: attention + projection per slot. Slot i's
                # projection is DEFERRED into slot i+1, emitted after its
                # first two score groups: the PE queue is in-order, so
                # projection matmuls waiting on PSUM-evacuation copies would
                # otherwise stall the queue at every slot boundary; here the
                # copies run while the next slot's scores/exp keep PE busy.
                pend_proj = None
                for i in range(NSLOT):
                    P0 = P0S[i]
                    ot = [
                        ps_at.tile([128, 512], F32, tag=f"ot{cc}", name=f"ot{cc}")
                        for cc in range(CC)
                    ]

                    def emit_scores(k):
                        w = _width(P0, k)
                        st = ps_at.tile([128, 512], F32, tag="st", bufs=3, name="st")
                        for cc in range(CC):
                            t0 = cc * 512 + (k % 4) * 128
                            nc.tensor.matmul(
                                st[:, 0 : w * 128],
                                lhsT=xts[:, k // 4, t0 : t0 + 128],
                                rhs=mq[:, cc, i * 512 : i * 512 + w * 128],
                                start=(cc == 0),
                                stop=(cc == CC - 1),
                            )
                        return st

                    # rowsums directly in row-partition layout: per 128-row
                    # group rt, rs[:, rt] = esum[:, rt-chunk].T @ ones.
                    # Own PSUM bank; each column is emitted mid-attention as
                    # soon as that rt's esum chunk takes its final add, so
                    # nothing rowsum-related sits on the slot-end path.
                    rs = ps_at.tile([128, 512], F32, tag="rs", name="rs")

                    # software pipeline, two deep: scores(k+1) AND (k+2)
                    # issue on PE before the exp(k)-dependent S matmuls.
                    # One score-group of shrinking width is too little PE
                    # cover for the ~900ns exp+mask chain at the slot tail.
                    st_q = [emit_scores(0)]
                    if P0 > 1:
                        st_q.append(emit_scores(1))
                    if pend_proj is not None:
                        pend_proj()
                        pend_proj = None
                    for k in range(P0):
                        st_cur = st_q.pop(0)
                        if k + 2 < P0:
                            st_q.append(emit_scores(k + 2))
                        w = _width(P0, k)
                        e = work.tile([128, 512], BF, tag="e", name="e")
                        # one exp over the full active width; the boundary
                        # chunk (always the last 128-tile, only in the final
                        # 8 key-steps of a slot) is then masked in place on
                        # DVE (faster turnaround than GpSimd, whose FIFO
                        # holds the esum accumulation).
                        nc.scalar.activation(
                            e[:, 0 : w * 128],
                            st_cur[:, 0 : w * 128],
                            mybir.ActivationFunctionType.Exp,
                        )
                        if k >= P0 - 8:
                            gb = i * 8 + (k - (P0 - 8))
                            nc.vector.tensor_tensor(
                                e[:, (w - 1) * 128 : w * 128],
                                e[:, (w - 1) * 128 : w * 128],
                                bm[:, gb, :],
                                op=mybir.AluOpType.mult,
                            )
                        # esum accumulation on GpSimd (k=0 initializes: w=4)
                        if k == 0:
                            nc.gpsimd.tensor_copy(esum[:], e[:])
                        else:
                            nc.gpsimd.tensor_tensor(
                                esum[:, 0 : w * 128],
                                esum[:, 0 : w * 128],
                                e[:, 0 : w * 128],
                                op=mybir.AluOpType.add,
                            )
                        # S[cx, row] += (x k-tile).T @ e
                        for cc in range(CC):
                            nc.tensor.matmul(
                                ot[cc][:, 0 : w * 128],
                                lhsT=xn[:, k, cc * 128 : (cc + 1) * 128],
                                rhs=e[:, 0 : w * 128],
                                start=(k == 0),
                                stop=(k == P0 - 1),
                            )
                        # rt = (P0+1-k)//2 took its final esum add two
                        # iterations ago; emitting its rowsum now lets the
                        # reorder window pull the LDWEIGHTS (dep already
                        # satisfied) behind the streaming matmuls
                        if P0 - 5 <= k <= P0 - 1 and (P0 + 1 - k) % 2 == 0:
                            rt = (P0 + 1 - k) // 2
                            nc.tensor.matmul(
                                rs[:, rt : rt + 1],
                                lhsT=esum[:, rt * 128 : (rt + 1) * 128],
                                rhs=ones[:],
                                start=True,
                                stop=True,
                            )

                    def pend_proj(i=i, ot=ot, rs=rs):
                        # rt=0's esum chunk finalized at k=P0-1; its rowsum
                        # and the reciprocal land here, off the critical path
                        nc.tensor.matmul(
                            rs[:, 0:1],
                            lhsT=esum[:, 0:128],
                            rhs=ones[:],
                            start=True,
                            stop=True,
                        )
                        nc.vector.reciprocal(rr[:, i * 4 : (i + 1) * 4], rs[:, 0:4])
                        # evacuate S with one whole-bank copy per cc chunk:
                        # fewer, larger ops halve the ACT/DVE copy time
                        otsb = work.tile([128, CC, 512], BF, tag="otsb", name="otsb")
                        for cc in range(CC):
                            if cc % 2 == 0:
                                nc.scalar.copy(otsb[:, cc, :], ot[cc][:])
                            else:
                                nc.vector.tensor_copy(otsb[:, cc, :], ot[cc][:])
                        for rt in range(4):
                            if i == NSLOT - 1 and rt == 3:
                                # the kernel's last output: two half-width
                                # groups in SEPARATE banks (same-bank halves
                                # would be a PSUM read/write collision), so
                                # scale-A (DVE) runs under the half-B
                                # matmuls and DMA-A under scale-B (ACT)
                                ysb = work.tile(
                                    [128, 512], F32, tag="ysb", name="ysb"
                                )
                                r0 = i * 512 + rt * 128
                                rrs = rr[:, i * 4 + rt : i * 4 + rt + 1]
                                for h, (h0, h1) in enumerate(
                                    [(0, 256), (256, 512)]
                                ):
                                    yph = ps_at.tile(
                                        [128, 512], F32, tag="st", bufs=3,
                                        name="yph",
                                    )
                                    for cc in range(CC):
                                        nc.tensor.matmul(
                                            yph[:, 0:256],
                                            lhsT=otsb[
                                                :, cc, rt * 128 : (rt + 1) * 128
                                            ],
                                            rhs=wpv[:, cc, h0:h1],
                                            start=(cc == 0),
                                            stop=(cc == CC - 1),
                                        )
                                    if h == 0:
                                        nc.vector.tensor_scalar(
                                            ysb[:, h0:h1],
                                            in0=yph[:, 0:256],
                                            scalar1=rrs,
                                            scalar2=None,
                                            op0=mybir.AluOpType.mult,
                                        )
                                    else:
                                        nc.scalar.activation(
                                            ysb[:, h0:h1],
                                            yph[:, 0:256],
                                            mybir.ActivationFunctionType.Copy,
                                            scale=rrs,
                                        )
                                    nc.sync.dma_start(
                                        d_y.ap()[r0 : r0 + 128, h0:h1],
                                        ysb[:, h0:h1],
                                    )
                                continue
                            # rides the st rotation (idle during projection):
                            # yp(rt+1) matmuls never wait for the DVE scale
                            # of yp(rt) to drain its bank
                            yp = ps_at.tile(
                                [128, 512], F32, tag="st", bufs=3, name="yp"
                            )
                            for cc in range(CC):
                                nc.tensor.matmul(
                                    yp[:],
                                    lhsT=otsb[:, cc, rt * 128 : (rt + 1) * 128],
                                    rhs=wpv[:, cc, :],
                                    start=(cc == 0),
                                    stop=(cc == CC - 1),
                                )
                            ysb = work.tile([128, 512], F32, tag="ysb", name="ysb")
                            # alternate engines: the st-bank rotation means
                            # the next slot's score groups wait on these
                            # reads; a single engine's FIFO serializes them
                            if rt % 2 == 0:
                                nc.vector.tensor_scalar(
                                    ysb[:],
                                    in0=yp[:],
                                    scalar1=rr[:, i * 4 + rt : i * 4 + rt + 1],
                                    scalar2=None,
                                    op0=mybir.AluOpType.mult,
                                )
                            else:
                                nc.scalar.activation(
                                    ysb[:],
                                    yp[:],
                                    mybir.ActivationFunctionType.Copy,
                                    scale=rr[:, i * 4 + rt : i * 4 + rt + 1],
                                )
                            r0 = i * 512 + rt * 128
                            nc.sync.dma_start(d_y.ap()[r0 : r0 + 128, :], ysb[:])

                pend_proj()

    _strip_mm_sem_incs(nc)
    _split_excess_waits(nc)
    return nc


_NC = None


def _get_program():
    global _NC
    if _NC is None:
        _NC = build_program()
    return _NC


LAST_RESULT = None


def kernel(x, w_qkv, w_proj):
    from concourse.bass_utils import run_bass_kernel_spmd

    x = np.asarray(x, dtype=np.float32)
    w_qkv = np.asarray(w_qkv, dtype=np.float32)
    w_proj = np.asarray(w_proj, dtype=np.float32)

    wq, wk, wv = w_qkv[0:C], w_qkv[C : 2 * C], w_qkv[2 * C : 3 * C]
    # scores^T = x M xq^T with M = Wk^T Wq / sqrt(C); kernel computes
    # mq = M xq^T via lhsT slices of M^T (layout [cx', cx])
    mTM = (wq.T @ wk) * SCALE  # = M^T  [cx', cx]
    wpvM = w_proj @ wv  # [o, cx]: y = D^-1 (E^T x) Wpv^T
    mT = np.ascontiguousarray(mTM).astype(BFNP)
    wpvT = np.ascontiguousarray(wpvM.T).astype(BFNP)

    # per-part boundary masks: slot i, boundary step bk (key-tile k =
    # P0-8+bk) masks the lowest-extent active tile, at slot position
    # lo = (7-bk)//2; mask[p, c] = row(tile, c) >= key(k, p). Part 0 sees
    # all-ones/triangular, part 1 triangular/all-zero in the same places.
    bms = {}
    for part in (0, 1):
        tiles = [t for slot in ASSIGN_TILES[part] for t in slot]
        bmp = np.zeros((128, NSLOT * 8, 128), dtype=np.float32)
        for i, P0 in enumerate(P0S):
            for bk in range(8):
                k = P0 - 8 + bk
                lo = (7 - bk) // 2
                t = tiles[4 * i + lo]
                rows = t * 128 + np.arange(128, dtype=np.int64)[None, :]
                keys = k * 128 + np.arange(128, dtype=np.int64)[:, None]
                bmp[:, i * 8 + bk, :] = (rows >= keys).astype(np.float32)
        bms[part] = np.ascontiguousarray(
            bmp.reshape(128, NSLOT * 8 * 128)
        ).astype(BFNP)

    # pre-tile weights into SBUF layouts:
    #   mTt[p, och, cc*256+oj] = mT[cc*128+p, och*256+oj]
    #   wpvTt[p, cc, o]        = wpvT[cc*128+p, o]
    mTt = np.ascontiguousarray(
        mT.reshape(4, 128, 2, 256).transpose(1, 2, 0, 3).reshape(128, 2, 1024)
    )
    wpvTt = np.ascontiguousarray(wpvT.reshape(4, 128, 512).transpose(1, 0, 2))

    in_maps = []
    for core in range(NCORES):
        b, part = divmod(core, 2)
        tiles = [t for slot in ASSIGN_TILES[part] for t in slot]
        xb = np.asarray(x[b]).astype(BFNP)  # [T, C]
        # xTt[p, kb, cc*512+t] = x[kb*512+t, cc*128+p]
        xTt = np.ascontiguousarray(
            xb.reshape(8, 512, 4, 128).transpose(3, 0, 2, 1).reshape(128, 8, 2048)
        )
        # xNt[p, tk, c] = x[tk*128+p, c]
        xNt = np.ascontiguousarray(xb.reshape(TK, 128, C).transpose(1, 0, 2))
        qx = np.concatenate(
            [xb[t * 128 : (t + 1) * 128, :] for t in tiles], 0
        )  # [QR, C]
        # qxTt[p, qb, cc*512+tl] = qx[qb*512+tl, cc*128+p]
        qxTt = qx.reshape(4, 512, 4, 128).transpose(3, 0, 2, 1).reshape(128, 8192)
        mqx = np.ascontiguousarray(
            np.concatenate([mTt.reshape(128, 2048), qxTt], axis=1)
        )
        in_maps.append(
            {
                "xTt": xTt,
                "xNt": xNt,
                "mqx": mqx,
                "wpvTt": wpvTt,
                "bm": bms[part],
            }
        )

    global LAST_RESULT
    res = run_bass_kernel_spmd(_get_program(), in_maps, core_ids=list(range(NCORES)))
    LAST_RESULT = res

    y = np.empty((B, T, C), dtype=np.float32)
    for core in range(NCORES):
        b, part = divmod(core, 2)
        yc = res.results[core]["y"]
        tiles = [t for slot in ASSIGN_TILES[part] for t in slot]
        for j, t in enumerate(tiles):
            y[b, t * 128 : (t + 1) * 128, :] = yc[j * 128 : (j + 1) * 128, :]
    return y

